# revision 75
# baseline (speedup 1.0000x reference)
"""MoE kernel for 8 TRN2 NeuronCores.

Strategy (expert-parallel, routing-as-sharding):
  - Router (Linear-GELU-Linear-softmax-top2) runs on host in f64 numpy;
    verified to reproduce the jax f32 reference top-2 sets exactly.
  - Token tiles (128 tokens, single expert each) are bin-packed onto the
    8 cores in up to two uniform "segments" per core: segment A runs sA
    tiles with one expert's weights, segment B runs sB tiles with a second
    expert's weights (loaded mid-kernel, overlapped with compute).
  - Per-core Bass kernel: 3-layer expert MLP with LayerNorm+exact-GELU
    between layers, bf16 matmuls with f32 PSUM accumulation, LN stats read
    PSUM directly, combine-weight scaling fused into output eviction.
    Software-pipelined across tiles (3-stage skew) to keep the PE busy.
  - Host scatter-adds the two expert contributions per token.
"""

import math
import os

import numpy as np

D, H, E, K = 512, 2048, 8, 2
EPS = 1e-5
P = 128

last_exec_time_ns = None


def _gelu_exact(x):
    from scipy.special import erf

    return 0.5 * x * (1.0 + erf(x / np.sqrt(2.0)))


def _route(t, Wg1, bg1, Wg2, bg2):
    th = t.astype(np.float64)
    h = th @ Wg1.astype(np.float64) + bg1.astype(np.float64)
    h = _gelu_exact(h)
    logits = h @ Wg2.astype(np.float64) + bg2.astype(np.float64)
    logits = logits - logits.max(axis=-1, keepdims=True)
    ex = np.exp(logits)
    gates = ex / ex.sum(axis=-1, keepdims=True)
    top2 = np.argsort(-gates, axis=-1, kind="stable")[:, :K]
    topv = np.take_along_axis(gates, top2, axis=-1)
    topv = topv / topv.sum(axis=-1, keepdims=True)
    return top2, topv.astype(np.float32)


def _pack_segments(tiles, n_slots=8):
    """Find minimal S and split S = sA + sB such that every expert's tile
    count can be covered by a_e A-slots (sA tiles each) + b_e B-slots (sB
    tiles each) with sum(a) <= n_slots, sum(b) <= n_slots.

    Returns (sA, sB, assign) where assign[e] = (a_e, b_e)."""
    total = sum(tiles)
    s_lo = max(1, (total + n_slots - 1) // n_slots)
    for S in range(s_lo, max(tiles) + 1):
        for sA in range(S, (S - 1) // 2, -1):
            sB = S - sA
            states = {(0, 0): 0}
            back = []
            ok = True
            for t in tiles:
                opts = []
                for a in range(n_slots + 1):
                    for b in range(n_slots + 1):
                        cap = a * sA + b * sB
                        if cap >= t:
                            opts.append((a, b, cap - t))
                new = {}
                for (au, bu), w in states.items():
                    for a, b, waste in opts:
                        if au + a <= n_slots and bu + b <= n_slots:
                            key = (au + a, bu + b)
                            val = (w + waste, (au, bu), (a, b))
                            if key not in new or new[key][0] > val[0]:
                                new[key] = val
                if not new:
                    ok = False
                    break
                back.append(new)
                states = {k: v[0] for k, v in new.items()}
            if not ok:
                continue
            key = min(states, key=lambda k: states[k])
            assign = []
            for st in reversed(back):
                w, prev, ab = st[key]
                assign.append(ab)
                key = prev
            return sA, sB, list(reversed(assign))
    return max(tiles), 0, [(1, 0)] * len(tiles)


def _build_program(n_a, n_b, affine, compute_dt_name="bfloat16", PD=5):
    """Per-core Bass program: n_a tiles with weight-set A, then n_b tiles
    with weight-set B (B weights streamed in mid-kernel).

    v2: LN rstd computed entirely on DVE (bit-trick seed + 2 Newton steps via
    the custom RECIPROCAL_APPROX_NR op) so the scalar engine only ever runs
    Gelu — the Sqrt<->Gelu activation-table reloads (~1.3us each, 4/tile) are
    gone. L2's PSUM chunks are drained to SBUF on DVE right after bn_stats
    (prologue L1 tiles too, since their gelus wait on the weight stream), so
    banks free early and a PD-deep prologue of L1 stages covers the 8MB W2
    startup DMA. Each tile's h1 transpose (PE + DVE drain into its own PSUM
    pool — keep it separate: sub-bank sharing with f32 accumulators corrupts)
    is emitted one iteration ahead of the consuming L2, spreading transpose
    ring pressure. Outputs store in two halves on alternating queues so the
    final tile's drain overlaps.

    v3 changes (trace-driven):
      - Startup stream over THREE queues (sync + gpsimd + scalar; scalar is
        idle until the first gelu ~15us in, so its HWDGE picks up the token
        prologue + W3a). First W1 chunk is k-split so the very first matmul
        only waits on 128KB of weights + tile-0 tokens.
      - Segment-B W1b/W3b loads deferred out of the startup window (issued
        at j==1, landing in the DMA-idle mid-kernel region).
      - W2b column-chunks 0/1 stream EARLY into the byte-identical dead
        spaces of w1a_s/w1b_s (a (P,4,2048) W1 tile and a (P,16,512) W2
        chunk are both 16KB/partition flat; flat offsets line up as
        k2*512+col == (k2//4)*2048 + (k2%4)*512 + col), as soon as the last
        L1 reader of each is emitted. Only chunks 2/3 stream into w2a_s at
        the j==n_a-1 swap, so the late swap is 4MB instead of 8MB and the
        first B tile's L2 starts on time.
      - 7-deep L1 prologue (bf16 hraw staging to fit SBUF); XBAR transposes
        on the sync queue for middle tiles only — first/last tiles use PE
        transposes (tail XBAR chain is serial; early XBARs would queue
        behind the startup stream)."""
    from concourse import bacc, bass, tile, mybir
    from concourse import masks
    from concourse.dve_ops import RECIPROCAL_APPROX_NR

    f32 = mybir.dt.float32
    u32 = mybir.dt.uint32
    bf16 = getattr(mybir.dt, compute_dt_name)
    AF = mybir.ActivationFunctionType
    ALU = mybir.AluOpType

    n_tiles = n_a + n_b
    C = n_tiles * P
    two_seg = n_b > 0
    NPRO = min(PD + 2, n_tiles)  # prologue depth (L1 tiles emitted up front)
    KD = D // P
    KH = H // P
    CS = 512
    NC1 = H // CS   # column-chunks per W1/W2
    HD = D // 2     # W3 column-half
    MAGIC_HALF = 0x5F3759DF - 0x400000  # rsqrt seed magic for hv = 0.5*(v+eps)

    nc = bacc.Bacc(None, target_bir_lowering=False, debug=False)

    # All weight/token dram tensors are partition-major AND chunk-major:
    # each DMA'd unit (a weight column-chunk, a token tile) is CONTIGUOUS
    # per partition. This is what sets descriptor counts: the DGE emits one
    # descriptor per contiguous run, and with the old (P, K, H) layout a
    # 2MB W2 chunk was 2048 x 1KB runs (~15us of descriptor generation —
    # the real startup bottleneck). Chunk-major makes it 128 x 16KB runs.
    #   W1: (P, NC1, KD, CS)   [c, k] -> W1[k*128+p, c*CS+col]
    #   W2: (P, NC1, KH, CS)
    #   W3: (P, 2, KH, HD)
    #   tT: (P, n_tiles, KD, P)
    #   tT is declared flat 2D: 4D slices like [:, 1:6] were NOT merged into
    #   big runs by the AP optimizer (640 x 1KB descriptors, ~85GB/s); a 2D
    #   contiguous slice is guaranteed one run per partition.
    TTW = KD * P  # columns per token tile
    tT_d = nc.dram_tensor("tT", (P, n_tiles * TTW), bf16, kind="ExternalInput")
    w1a_d = nc.dram_tensor("W1a", (P, NC1, KD, CS), bf16, kind="ExternalInput")
    w2a_d = nc.dram_tensor("W2a", (P, NC1, KH, CS), bf16, kind="ExternalInput")
    w3a_d = nc.dram_tensor("W3a", (P, 2, KH, HD), bf16, kind="ExternalInput")
    cw_d = nc.dram_tensor("cw", (P, n_tiles), f32, kind="ExternalInput")
    out_d = nc.dram_tensor("out", (C, D), f32, kind="ExternalOutput")
    if two_seg:
        w1b_d = nc.dram_tensor("W1b", (P, NC1, KD, CS), bf16, kind="ExternalInput")
        w2b_d = nc.dram_tensor("W2b", (P, NC1, KH, CS), bf16, kind="ExternalInput")
        w3b_d = nc.dram_tensor("W3b", (P, 2, KH, HD), bf16, kind="ExternalInput")

    aff_d = {}
    for name, width in (
        ("b1", H), ("g1", H), ("be1", H),
        ("b2", H), ("g2", H), ("be2", H),
        ("b3", D),
    ):
        if affine[name]:
            aff_d[name] = nc.dram_tensor(name, (P, width), f32, kind="ExternalInput")

    with tile.TileContext(nc) as tc:
        with (
            tc.tile_pool(name="const", bufs=1) as const_pool,
            tc.tile_pool(name="hraw", bufs=2) as hraw_pool,
            tc.tile_pool(name="xg", bufs=2) as xg_pool,
            tc.tile_pool(name="hT", bufs=2) as hT_pool,
            tc.tile_pool(name="outp", bufs=2) as out_pool,
            tc.tile_pool(name="st", bufs=4) as st_pool,
            tc.tile_pool(name="acc", bufs=6, space="PSUM") as acc_pool,
            tc.tile_pool(name="tp", bufs=2, space="PSUM") as tp_pool,
        ):
            # ---- resident loads (segment A + shared) ----
            w1a_s = const_pool.tile((P, NC1, KD, CS), bf16)
            w2a_s = const_pool.tile((P, NC1, KH, CS), bf16)
            w3a_s = const_pool.tile((P, 2, KH, HD), bf16)
            tT_s = const_pool.tile((P, n_tiles * TTW), bf16)
            cw_s = const_pool.tile((P, n_tiles), f32)
            # Startup stream over the two independent DGE paths: sync (the
            # hardware DGE — NOTE scalar shares this same HW DGE, so a
            # scalar-queue DMA adds no bandwidth, it just steals from sync)
            # and gpsimd (software DGE). The W2-complete time gates tile 0's
            # L2, so everything not needed before ~45us (W3a, cw, trailing
            # tokens) goes BEHIND W2 on the queues. Segment-B weights are
            # NOT loaded here (deferred to j==1).
            # NOTE a DMA instruction only pushes descriptors (it completes in
            # <1us); per-RING service order == emission order, but the two
            # rings share the fabric. So: per-ring FIFO is the scheduler, and
            # the byte-prefix ahead of each item on its ring (plus the other
            # ring's concurrent load) sets its arrival time. Both rings are
            # balanced so all four W2 chunks land by ~41us; W3a (split in
            # halves across rings), cw and the segment-B W1/W3 go behind W2.
            # W2 c0 and c3 are split in k-halves across both rings: arrivals
            # then pace out as c0@~22, c1@~33, c2@~36, c3@~42, each ahead of
            # L2(0)'s 3.4us/chunk consumption which starts ~38 after the
            # 7-tile L1 prologue.
            PDp = PD + 1
            nc.sync.dma_start(tT_s[:, :TTW], tT_d[:, :TTW])
            nc.gpsimd.dma_start(w1a_s[:, 2], w1a_d[:, 2])
            nc.sync.dma_start(w1a_s[:, 0], w1a_d[:, 0])
            nc.gpsimd.dma_start(w1a_s[:, 3], w1a_d[:, 3])
            nc.sync.dma_start(w1a_s[:, 1], w1a_d[:, 1])
            nc.gpsimd.dma_start(tT_s[:, PDp * TTW:], tT_d[:, PDp * TTW:])
            nc.sync.dma_start(tT_s[:, TTW:PDp * TTW], tT_d[:, TTW:PDp * TTW])
            for c, q in ((0, nc.sync), (1, nc.gpsimd), (2, nc.sync), (3, nc.gpsimd)):
                q.dma_start(w2a_s[:, c], w2a_d[:, c])
            nc.sync.dma_start(w3a_s[:, 0], w3a_d[:, 0])
            nc.gpsimd.dma_start(w3a_s[:, 1], w3a_d[:, 1])
            nc.gpsimd.dma_start(cw_s[:], cw_d[:])
            if two_seg:
                w1b_s = const_pool.tile((P, NC1, KD, CS), bf16)
                w3b_s = const_pool.tile((P, 2, KH, HD), bf16)
                nc.sync.dma_start(w1b_s[:], w1b_d[:])
                nc.gpsimd.dma_start(w3b_s[:], w3b_d[:])

            identity = const_pool.tile((P, P), bf16)
            masks.make_identity(nc, identity[:])

            magic_t = const_pool.tile((P, 1), u32, name="magic_t")
            nc.vector.memset(magic_t[:], MAGIC_HALF)

            aff_s = {}
            for name in aff_d:
                width = aff_d[name].shape[1]
                row = const_pool.tile((P, width), f32, name=f"{name}_bcast")
                nc.sync.dma_start(row[:], aff_d[name][:])
                aff_s[name] = row

            def weights_for(i):
                if (not two_seg) or i < n_a:
                    return w1a_s, w2a_s, w3a_s
                return w1b_s, w2a_s, w3b_s

            # Can W2b's c1 chunk live in w1b_s? Only if every B tile's L1
            # (the w1b readers) is emitted before the c1 stream must go out
            # (i.e. before the first B tile's L2 at j == n_a).
            c1_in_w1b = two_seg and n_b <= NPRO - 1

            def w2_rhs_for(i, k, c):
                """Streaming rhs for L2 chunk c, k-block k of tile i.

                B tiles read chunk 0 from w1a_s's space and (usually) chunk 1
                from w1b_s's space: a (P,4,4,512) W1 tile and a (P,16,512) W2
                column-chunk are both 16KB/partition flat and line up as
                k*512+col == (k//4)*2048 + (k%4)*512 + col, and the W1
                spaces are dead once all L1s have run — so W2b c0/c1 stream
                there mid-kernel instead of in the late j==n_a-1 swap."""
                if (not two_seg) or i < n_a:
                    return w2a_s[:, c, k, :]
                if c == 0:
                    return w1a_s[:, k // KD, k % KD, :]
                if c == 1 and c1_in_w1b:
                    return w1b_s[:, k // KD, k % KD, :]
                return w2a_s[:, c, k, :]

            def dve_rsqrt(var_ap, eng=None):
                """rstd = 1/sqrt(var+eps) without an ACT-table switch.
                Seed y = bitcast(MAGIC_HALF - (bits(hv)>>1)) with hv =
                0.5*(var+eps), then two Newton steps y*(1.5 - hv*y^2).
                eng=nc.vector uses the fused custom DVE op; eng=nc.gpsimd
                runs the same math with plain ALU ops on the (prologue-idle)
                Pool engine so DVE keeps up with the 7-tile L1 prologue."""
                eng = eng or nc.vector
                on_dve = eng is nc.vector
                hv = st_pool.tile((P, 1), f32, tag="hv")
                eng.tensor_scalar(
                    out=hv[:], in0=var_ap, scalar1=0.5, scalar2=EPS * 0.5,
                    op0=ALU.mult, op1=ALU.add,
                )
                rstd = st_pool.tile((P, 1), f32, tag="rstd")
                eng.tensor_scalar(
                    out=rstd[:].bitcast(u32), in0=hv[:].bitcast(u32),
                    scalar1=1, scalar2=None, op0=ALU.logical_shift_right,
                )
                eng.tensor_tensor(
                    out=rstd[:].bitcast(u32), in0=magic_t[:],
                    in1=rstd[:].bitcast(u32), op=ALU.subtract,
                )
                nt = st_pool.tile((P, 1), f32, tag="nt")
                for _ in range(2):
                    eng.tensor_tensor(
                        out=nt[:], in0=hv[:], in1=rstd[:], op=ALU.mult
                    )
                    if on_dve:
                        nc.vector._custom_dve(
                            RECIPROCAL_APPROX_NR, out=rstd[:], in0=nt[:],
                            in1=rstd[:], s0=1.5,
                        )
                    else:
                        # y*(1.5 - nt*y) via plain ops: t = nt*y; t = 1.5-t;
                        # y = y*t
                        eng.tensor_tensor(
                            out=nt[:], in0=nt[:], in1=rstd[:], op=ALU.mult
                        )
                        eng.tensor_scalar(
                            out=nt[:], in0=nt[:], scalar1=-1.0, scalar2=1.5,
                            op0=ALU.mult, op1=ALU.add,
                        )
                        eng.tensor_tensor(
                            out=rstd[:], in0=rstd[:], in1=nt[:], op=ALU.mult
                        )
                return rstd

            def mm_ln_gelu(tile_i, lhsT_getter, n_k, rhs_get, nh, bname, gname, bename, xg_tag,
                           filler_after=None, drain_l1=False):
                """matmul (-> +b) -> LN -> (*g +be) -> gelu; returns xg tile.

                rhs_get(k, c) -> streaming-operand AP for k-block k, chunk c.
                filler_after: {chunk_idx: fn} — emit fn() after that chunk's
                matmuls (PE filler while a DMA-paced weight column streams)."""
                nch = nh // CS
                fast = not (affine[bname] or affine[gname] or affine[bename])
                # bf16 staging: only ever a gelu input (stats read f32 PSUM
                # directly); halves the pool so a deeper prologue fits SBUF
                hraw = hraw_pool.tile((P, nh), bf16 if fast else f32, tag="hraw")
                stats = st_pool.tile((P, nch, 6), f32, tag="stats")
                ps_list = []
                for c in range(nch):
                    ps = acc_pool.tile((P, CS), f32, name="ps_acc", tag="ps_acc")
                    for k in range(n_k):
                        nc.tensor.matmul(
                            ps[:],
                            lhsT_getter(k),
                            rhs_get(k, c),
                            start=(k == 0),
                            stop=(k == n_k - 1),
                        )
                    cs_sl = slice(c * CS, (c + 1) * CS)
                    if fast:
                        # stats read PSUM. Steady-state L1 keeps its PSUM
                        # until the gelu (no drain — keeps DVE free for the
                        # hT copies that gate the transpose ring); L2 drains
                        # to SBUF on DVE so banks free early. Prologue L1s
                        # drain FIRST (alternating ACT/DVE so neither engine
                        # backpressures PSUM recycling) and compute stats
                        # LATER in one wide 2048-col bn_stats over hraw
                        # (~1.3us vs 2.8us of per-chunk PSUM stats on DVE).
                        # GPSIMD cannot read PSUM on real HW.
                        if drain_l1 and xg_tag == "xg1":
                            if c % 2 == 0:
                                nc.scalar.copy(hraw[:, cs_sl], ps[:])
                            else:
                                nc.vector.tensor_copy(hraw[:, cs_sl], ps[:])
                        elif xg_tag == "xg1":
                            nc.vector.bn_stats(stats[:, c, :], ps[:])
                            ps_list.append(ps)
                        else:
                            nc.vector.bn_stats(stats[:, c, :], ps[:])
                            nc.vector.tensor_copy(hraw[:, cs_sl], ps[:])
                    else:
                        nc.scalar.copy(hraw[:, cs_sl], ps[:])
                        if affine[bname]:
                            nc.vector.tensor_tensor(
                                out=hraw[:, cs_sl], in0=hraw[:, cs_sl],
                                in1=aff_s[bname][:, cs_sl], op=ALU.add,
                            )
                        nc.vector.bn_stats(stats[:, c, :], hraw[:, cs_sl])
                    if filler_after and c in filler_after:
                        filler_after[c]()
                mv = st_pool.tile((P, 2), f32, tag="mv")
                if fast and drain_l1 and xg_tag == "xg1":
                    # stats from the drained bf16 hraw: 2x DVE throughput
                    # vs f32 PSUM reads (bn_stats is capped at 512 wide)
                    for c in range(nch):
                        nc.vector.bn_stats(
                            stats[:, c, :], hraw[:, c * CS:(c + 1) * CS]
                        )
                nc.vector.bn_aggr(mv[:], stats[:])
                # (Pool engine rejects TensorScalar at the ISA level, so the
                # whole LN chain stays on DVE)
                ln_eng = nc.vector
                rstd = dve_rsqrt(mv[:, 1:2], eng=ln_eng)
                negmr = st_pool.tile((P, 1), f32, tag="negmr")
                ln_eng.tensor_scalar(
                    out=negmr[:], in0=mv[:, 0:1], scalar1=rstd[:], scalar2=-1.0,
                    op0=ALU.mult, op1=ALU.mult,
                )
                xg = xg_pool.tile(
                    (P, nh), bf16, tag=xg_tag,
                    bufs=(NPRO if xg_tag == "xg1" else 2),
                )
                for c in range(nch):
                    cs_sl = slice(c * CS, (c + 1) * CS)
                    if fast:
                        # first piece of chunk 0 is narrow so the first PE
                        # transpose of this xg unblocks as early as possible
                        pieces = [(0, P), (P, CS)] if c == 0 else [(0, CS)]
                        for lo, hi in pieces:
                            if xg_tag == "xg1" and not drain_l1:
                                in_ap = ps_list[c][:, lo:hi]
                            else:
                                in_ap = hraw[:, c * CS + lo:c * CS + hi]
                            nc.scalar.activation(
                                xg[:, c * CS + lo:c * CS + hi], in_ap, AF.Gelu,
                                bias=negmr[:], scale=rstd[:],
                            )
                    else:
                        xn = hraw_pool.tile((P, CS), f32, name="xn", tag="xn")
                        nc.vector.tensor_scalar(
                            out=xn[:], in0=hraw[:, cs_sl],
                            scalar1=mv[:, 0:1], scalar2=rstd[:],
                            op0=ALU.subtract, op1=ALU.mult,
                        )
                        if affine[gname]:
                            nc.vector.tensor_tensor(
                                out=xn[:], in0=xn[:], in1=aff_s[gname][:, cs_sl],
                                op=ALU.mult,
                            )
                        if affine[bename]:
                            nc.vector.tensor_tensor(
                                out=xn[:], in0=xn[:], in1=aff_s[bename][:, cs_sl],
                                op=ALU.add,
                            )
                        nc.scalar.activation(xg[:, cs_sl], xn[:], AF.Gelu)
                return xg

            def transpose_to_hT(xg, nh, hT_tag, use_xbar=False, xq=None):
                """PE-transpose (P, nh) bf16 -> (P, nh//P, P) feature-major.

                hT1 (b-stage) drains on DVE, hT2 (c-stage) on ACT so neither
                engine's queue delays the other stage's PSUM->SBUF handoff."""
                nch = nh // CS
                hT = hT_pool.tile((P, nh // P, P), bf16, tag=hT_tag)
                if use_xbar:
                    # XBAR DMA transpose: hT[p,k,q] = xg[q,128k+p], ~1.8us on
                    # the DMA engine. Issued on the SYNC queue, not scalar:
                    # a DMA instruction occupies its engine until the wait
                    # clears, and on scalar that blocks the gelu stream.
                    # Sync mid-kernel only carries slack-tolerant stores and
                    # the W2b streams. Not used for early tiles whose XBAR
                    # would queue behind the 13MB startup stream.
                    (xq or nc.sync).dma_start_transpose(hT[:], xg[:])
                    return hT
                for c in range(nch):
                    pt = tp_pool.tile((P, CS), bf16, name="pt", tag="pt")
                    for j in range(CS // P):
                        b = c * (CS // P) + j
                        nc.tensor.transpose(
                            pt[:, j * P:(j + 1) * P],
                            xg[:, b * P:(b + 1) * P],
                            identity[:],
                        )
                    nc.vector.tensor_copy(
                        hT[:, c * (CS // P):(c + 1) * (CS // P), :], pt[:]
                    )
                return hT

            xg1 = {}
            xg2 = {}
            hT1 = {}
            hT2 = {}

            def stage_a(i):
                w1_s = weights_for(i)[0]
                xg1[i] = mm_ln_gelu(
                    i, lambda k: tT_s[:, i * TTW + k * P:i * TTW + (k + 1) * P],
                    KD, lambda k, c: w1_s[:, c, k, :], H,
                    "b1", "g1", "be1", "xg1", drain_l1=(i < NPRO - 1),
                )

            def stage_b(i, filler=None):
                h1T = hT1.pop(i)
                xg2[i] = mm_ln_gelu(
                    i, lambda k: h1T[:, k, :], KH,
                    lambda k, c: w2_rhs_for(i, k, c), H,
                    "b2", "g2", "be2", "xg2", filler_after=filler,
                )
                # last tile's chain gelu->XBAR->L3 is serial (nothing left
                # to overlap) while PE transposes pipeline per-chunk: keep
                # the tail (and startup-stream-blocked tile 0) on the PE
                # tail tiles use PE transposes: the XBAR path's chain
                # (all gelus -> whole-tile XBAR -> L3) is serial when no
                # other PE work remains, while PE transposes pipeline
                # per-chunk with the gelus
                hT2[i] = transpose_to_hT(xg2.pop(i), H, "hT2",
                                         use_xbar=(1 <= i <= n_tiles - 2))

            def stage_c(i):
                w3_s = weights_for(i)[2]
                h2T = hT2.pop(i)
                tok = slice(i * P, (i + 1) * P)
                outt = out_pool.tile((P, D), f32, tag="outt")
                # two D/2 halves: the first half's evict+store overlaps the
                # second half's matmuls (shrinks the end-of-kernel drain)
                for h in range(2):
                    dsl = slice(h * HD, (h + 1) * HD)
                    ps3 = acc_pool.tile((P, HD), f32, name="ps3", tag="ps_acc")
                    for k in range(KH):
                        nc.tensor.matmul(
                            ps3[:], h2T[:, k, :], w3_s[:, h, k, :],
                            start=(k == 0), stop=(k == KH - 1),
                        )
                    if affine["b3"]:
                        nc.vector.tensor_tensor(
                            out=outt[:, dsl], in0=ps3[:], in1=aff_s["b3"][:, dsl],
                            op=ALU.add,
                        )
                        nc.scalar.mul(outt[:, dsl], outt[:, dsl], cw_s[:, i:i + 1])
                    else:
                        nc.vector.tensor_scalar(
                            out=outt[:, dsl], in0=ps3[:],
                            scalar1=cw_s[:, i:i + 1], scalar2=None,
                            op0=ALU.mult, op1=ALU.bypass,
                        )
                    # alternate queues so the two stores overlap (matters for
                    # the end-of-kernel drain of the final tile)
                    (nc.gpsimd, nc.sync)[h].dma_start(out_d[tok, dsl], outt[:, dsl])

            # prologue: queue NPRO stage-A tiles so the PE has L1 work while
            # the 8MB W2 load is still streaming in (all but the last drain
            # their PSUM to SBUF; the last prologue tile keeps PSUM)
            next_a = NPRO
            for i in range(next_a):
                stage_a(i)
                # interleave the first two hT1 transposes into the prologue:
                # their DVE drains then run BEFORE the later tiles' LN
                # chains in the in-order DVE stream, so hT1(0) (which gates
                # L2(0)) is ready ~5us earlier
                if i == 2:
                    hT1[0] = transpose_to_hT(xg1.pop(0), H, "hT1")
                if i == 3 and n_tiles > 1:
                    hT1[1] = transpose_to_hT(xg1.pop(1), H, "hT1")
            if 0 not in hT1:
                hT1[0] = transpose_to_hT(xg1.pop(0), H, "hT1")

            w2b_c0_done = not two_seg
            w2b_c1_done = not (two_seg and c1_in_w1b)
            for j in range(n_tiles):
                if j + 1 < n_tiles and j + 1 not in hT1:
                    hT1[j + 1] = transpose_to_hT(
                        xg1.pop(j + 1), H, "hT1",
                        use_xbar=(2 <= j + 1 <= n_tiles - 2),
                    )
                stage_b(j)
                # B-tile L1s are additionally held until j >= 2 so their
                # w1b dependency (ring tail, ~55us) can't stall the queue
                if (next_a < n_tiles and next_a <= j + NPRO
                        and (next_a < n_a or j >= 2)):
                    stage_a(next_a)
                    next_a += 1
                # W2b c0 (and usually c1) stream into the dead W1 spaces.
                # CAREFUL: a DMA instruction's semaphore wait BLOCKS its
                # whole in-order queue (out-stores behind it → outt-buffer
                # recycling → PE stall), so emit these only at a j where the
                # wait (all L1 readers of that W1 space done) will already
                # have cleared when the instruction reaches the queue head
                # (the queue head trails by ~2 tiles of out-stores).
                if (two_seg and not w2b_c0_done and next_a > n_a - 1
                        and j >= n_a - 3):
                    nc.gpsimd.dma_start(w1a_s[:], w2b_d[:, 0])
                    w2b_c0_done = True
                if (not w2b_c1_done and next_a > n_tiles - 1
                        and j >= n_a - 2):
                    nc.sync.dma_start(w1b_s[:], w2b_d[:, 1])
                    w2b_c1_done = True
                # remaining W2b chunks overwrite w2a_s once their last
                # segment-A L2 readers are issued (the 4MB late swap)
                if two_seg and j == n_a - 1:
                    late = [2, 3] if c1_in_w1b else [1, 2, 3]
                    for c in late:
                        q = (nc.sync, nc.gpsimd)[c % 2]
                        q.dma_start(w2a_s[:, c], w2b_d[:, c])
                if j >= 1:
                    stage_c(j - 1)
            stage_c(n_tiles - 1)

    nc.compile()
    return nc


def kernel(**inputs):
    global last_exec_time_ns
    import ml_dtypes

    from concourse import bass_utils

    inp = {k: np.asarray(v) for k, v in inputs.items()}
    x = inp["x"].astype(np.float32, copy=False)
    B, S, d = x.shape
    T = B * S
    t = x.reshape(T, d)

    top2, topv = _route(t, inp["Wg1"], inp["bg1"], inp["Wg2"], inp["bg2"])

    idx_per_e = []
    w_per_e = []
    for e in range(E):
        sel = np.nonzero(top2 == e)
        idx_per_e.append(sel[0])
        w_per_e.append(topv[sel].astype(np.float32))

    affine = {
        "b1": not np.all(inp["b1"] == 0.0),
        "g1": not np.all(inp["g1"] == 1.0),
        "be1": not np.all(inp["be1"] == 0.0),
        "b2": not np.all(inp["b2"] == 0.0),
        "g2": not np.all(inp["g2"] == 1.0),
        "be2": not np.all(inp["be2"] == 0.0),
        "b3": not np.all(inp["b3"] == 0.0),
    }
    any_affine = any(affine.values())

    tiles_e = [int(math.ceil(len(ix) / P)) for ix in idx_per_e]
    if any_affine:
        # affine params are per-expert; keep one expert per core
        sA, sB = max(max(tiles_e), 1), 0
        assign = [(1, 0)] * E
    else:
        sA, sB, assign = _pack_segments(tiles_e)

    # build slot lists: each slot = (expert, first_piece, n_pieces)
    slotsA, slotsB = [], []
    for e in range(E):
        a_e, b_e = assign[e]
        pos = 0
        nt = tiles_e[e]
        for _ in range(a_e):
            take = max(0, min(sA, nt - pos))
            slotsA.append((e, pos, take))
            pos += take
        for _ in range(b_e):
            take = max(0, min(sB, nt - pos))
            slotsB.append((e, pos, take))
            pos += take
    while len(slotsA) < E:
        slotsA.append((None, 0, 0))
    while len(slotsB) < E:
        slotsB.append((None, 0, 0))

    n_tiles = sA + sB
    C = n_tiles * P
    CS = 512
    bf = ml_dtypes.bfloat16
    # chunk-major layouts (see _build_program): each DMA'd unit contiguous
    zW1 = np.zeros((P, H // CS, D // P, CS), bf)
    zW2 = np.zeros((P, H // CS, H // P, CS), bf)
    zW3 = np.zeros((P, 2, H // P, D // 2), bf)

    def slot_tokens(slot, s_cap):
        """token columns (D, s_cap*P) f32 + cw (s_cap*P,) for one slot."""
        e, pos, take = slot
        tt = np.zeros((D, s_cap * P), np.float32)
        cw = np.zeros((s_cap * P,), np.float32)
        if e is not None and take > 0:
            lo = pos * P
            hi = min(len(idx_per_e[e]), (pos + take) * P)
            n = hi - lo
            tt[:, :n] = t[idx_per_e[e][lo:hi]].T
            cw[:n] = w_per_e[e][lo:hi]
        return tt, cw

    def expert_w(e, which):
        if e is None:
            return (zW1, zW2, zW3)[which]
        w = (inp["W1"], inp["W2"], inp["W3"])[which][e]
        kk = (D // P, H // P, H // P)[which]
        cw_ = (CS, CS, D // 2)[which]
        nch = w.shape[1] // cw_
        # partition-major AND chunk-major: (P, nch, kk, cw) — the DMA'd
        # unit (one column-chunk) is contiguous per partition
        return np.ascontiguousarray(
            np.asarray(w).reshape(kk, P, nch, cw_).transpose(1, 2, 0, 3).astype(bf)
        )

    in_maps = []
    for c in range(E):
        ttA, cwA = slot_tokens(slotsA[c], sA)
        eA = slotsA[c][0]
        if sB > 0:
            ttB, cwB = slot_tokens(slotsB[c], sB)
            tt = np.concatenate([ttA, ttB], axis=1)
            cw = np.concatenate([cwA, cwB])
        else:
            tt, cw = ttA, cwA
        m = {
            # flat (P, n_tiles*KD*P): one token tile contiguous per partition
            "tT": np.ascontiguousarray(
                tt.reshape(D // P, P, n_tiles, P).transpose(1, 2, 0, 3)
            ).astype(bf).reshape(P, -1),
            "W1a": expert_w(eA, 0),
            "W2a": expert_w(eA, 1),
            "W3a": expert_w(eA, 2),
            "cw": np.ascontiguousarray(cw.reshape(n_tiles, P).T).astype(np.float32),
        }
        if sB > 0:
            eB = slotsB[c][0]
            m["W1b"] = expert_w(eB, 0)
            m["W2b"] = expert_w(eB, 1)
            m["W3b"] = expert_w(eB, 2)
        for name in ("b1", "g1", "be1", "b2", "g2", "be2", "b3"):
            if affine[name]:
                row = np.asarray(inp[name][eA if eA is not None else 0], np.float32)
                m[name] = np.ascontiguousarray(np.broadcast_to(row, (P, row.shape[0])))
        in_maps.append(m)

    nc = _build_program(sA, sB, affine)

    trace = bool(os.environ.get("KERNEL_TRACE"))
    if trace:
        try:
            from antenv import axon_hooks as _ah  # noqa: F401
        except ImportError:
            trace = False
    try:
        res = bass_utils.run_bass_kernel_spmd(
            nc, in_maps, core_ids=list(range(E)), trace=trace
        )
    except Exception:
        if not trace:
            raise
        res = bass_utils.run_bass_kernel_spmd(
            nc, in_maps, core_ids=list(range(E)), trace=False
        )
    last_exec_time_ns = getattr(res, "exec_time_ns", None)

    out_full = np.zeros((T, D), np.float32)
    for c in range(E):
        o = np.asarray(res.results[c]["out"], np.float32)
        for si, (slot, s_cap, base) in enumerate(
            ((slotsA[c], sA, 0), (slotsB[c], sB, sA * P))
        ):
            e, pos, take = slot
            if e is None or take == 0:
                continue
            lo = pos * P
            hi = min(len(idx_per_e[e]), (pos + take) * P)
            n = hi - lo
            out_full[idx_per_e[e][lo:hi]] += o[base:base + n]
    return out_full.reshape(B, S, D).astype(np.float32)



# revision 77
# speedup vs baseline: 1.0139x; 1.0139x over previous
"""MoE kernel for 8 TRN2 NeuronCores.

Strategy (expert-parallel, routing-as-sharding):
  - Router (Linear-GELU-Linear-softmax-top2) runs on host in f64 numpy;
    verified to reproduce the jax f32 reference top-2 sets exactly.
  - Token tiles (128 tokens, single expert each) are bin-packed onto the
    8 cores in up to two uniform "segments" per core: segment A runs sA
    tiles with one expert's weights, segment B runs sB tiles with a second
    expert's weights (loaded mid-kernel, overlapped with compute).
  - Per-core Bass kernel: 3-layer expert MLP with LayerNorm+exact-GELU
    between layers, bf16 matmuls with f32 PSUM accumulation, LN stats read
    PSUM directly, combine-weight scaling fused into output eviction.
    Software-pipelined across tiles (3-stage skew) to keep the PE busy.
  - Host scatter-adds the two expert contributions per token.
"""

import math
import os

import numpy as np

D, H, E, K = 512, 2048, 8, 2
EPS = 1e-5
P = 128

last_exec_time_ns = None


def _gelu_exact(x):
    from scipy.special import erf

    return 0.5 * x * (1.0 + erf(x / np.sqrt(2.0)))


def _route(t, Wg1, bg1, Wg2, bg2):
    th = t.astype(np.float64)
    h = th @ Wg1.astype(np.float64) + bg1.astype(np.float64)
    h = _gelu_exact(h)
    logits = h @ Wg2.astype(np.float64) + bg2.astype(np.float64)
    logits = logits - logits.max(axis=-1, keepdims=True)
    ex = np.exp(logits)
    gates = ex / ex.sum(axis=-1, keepdims=True)
    top2 = np.argsort(-gates, axis=-1, kind="stable")[:, :K]
    topv = np.take_along_axis(gates, top2, axis=-1)
    topv = topv / topv.sum(axis=-1, keepdims=True)
    return top2, topv.astype(np.float32)


def _pack_segments(tiles, n_slots=8):
    """Find minimal S and split S = sA + sB such that every expert's tile
    count can be covered by a_e A-slots (sA tiles each) + b_e B-slots (sB
    tiles each) with sum(a) <= n_slots, sum(b) <= n_slots.

    Returns (sA, sB, assign) where assign[e] = (a_e, b_e)."""
    total = sum(tiles)
    s_lo = max(1, (total + n_slots - 1) // n_slots)
    for S in range(s_lo, max(tiles) + 1):
        for sA in range(S, (S - 1) // 2, -1):
            sB = S - sA
            states = {(0, 0): 0}
            back = []
            ok = True
            for t in tiles:
                opts = []
                for a in range(n_slots + 1):
                    for b in range(n_slots + 1):
                        cap = a * sA + b * sB
                        if cap >= t:
                            opts.append((a, b, cap - t))
                new = {}
                for (au, bu), w in states.items():
                    for a, b, waste in opts:
                        if au + a <= n_slots and bu + b <= n_slots:
                            key = (au + a, bu + b)
                            val = (w + waste, (au, bu), (a, b))
                            if key not in new or new[key][0] > val[0]:
                                new[key] = val
                if not new:
                    ok = False
                    break
                back.append(new)
                states = {k: v[0] for k, v in new.items()}
            if not ok:
                continue
            key = min(states, key=lambda k: states[k])
            assign = []
            for st in reversed(back):
                w, prev, ab = st[key]
                assign.append(ab)
                key = prev
            return sA, sB, list(reversed(assign))
    return max(tiles), 0, [(1, 0)] * len(tiles)


def _build_program(n_a, n_b, affine, compute_dt_name="bfloat16", PD=5):
    """Per-core Bass program: n_a tiles with weight-set A, then n_b tiles
    with weight-set B (B weights streamed in mid-kernel).

    v2: LN rstd computed entirely on DVE (bit-trick seed + 2 Newton steps via
    the custom RECIPROCAL_APPROX_NR op) so the scalar engine only ever runs
    Gelu — the Sqrt<->Gelu activation-table reloads (~1.3us each, 4/tile) are
    gone. L2's PSUM chunks are drained to SBUF on DVE right after bn_stats
    (prologue L1 tiles too, since their gelus wait on the weight stream), so
    banks free early and a PD-deep prologue of L1 stages covers the 8MB W2
    startup DMA. Each tile's h1 transpose (PE + DVE drain into its own PSUM
    pool — keep it separate: sub-bank sharing with f32 accumulators corrupts)
    is emitted one iteration ahead of the consuming L2, spreading transpose
    ring pressure. Outputs store in two halves on alternating queues so the
    final tile's drain overlaps.

    v3 changes (trace-driven):
      - Startup stream over THREE queues (sync + gpsimd + scalar; scalar is
        idle until the first gelu ~15us in, so its HWDGE picks up the token
        prologue + W3a). First W1 chunk is k-split so the very first matmul
        only waits on 128KB of weights + tile-0 tokens.
      - Segment-B W1b/W3b loads deferred out of the startup window (issued
        at j==1, landing in the DMA-idle mid-kernel region).
      - W2b column-chunks 0/1 stream EARLY into the byte-identical dead
        spaces of w1a_s/w1b_s (a (P,4,2048) W1 tile and a (P,16,512) W2
        chunk are both 16KB/partition flat; flat offsets line up as
        k2*512+col == (k2//4)*2048 + (k2%4)*512 + col), as soon as the last
        L1 reader of each is emitted. Only chunks 2/3 stream into w2a_s at
        the j==n_a-1 swap, so the late swap is 4MB instead of 8MB and the
        first B tile's L2 starts on time.
      - 7-deep L1 prologue (bf16 hraw staging to fit SBUF); XBAR transposes
        on the sync queue for middle tiles only — first/last tiles use PE
        transposes (tail XBAR chain is serial; early XBARs would queue
        behind the startup stream)."""
    from concourse import bacc, bass, tile, mybir
    from concourse import masks
    from concourse.dve_ops import RECIPROCAL_APPROX_NR

    f32 = mybir.dt.float32
    u32 = mybir.dt.uint32
    bf16 = getattr(mybir.dt, compute_dt_name)
    AF = mybir.ActivationFunctionType
    ALU = mybir.AluOpType

    n_tiles = n_a + n_b
    C = n_tiles * P
    two_seg = n_b > 0
    NPRO = min(PD + 2, n_tiles)  # prologue depth (L1 tiles emitted up front)
    KD = D // P
    KH = H // P
    CS = 512
    NC1 = H // CS   # column-chunks per W1/W2
    HD = D // 2     # W3 column-half
    MAGIC_HALF = 0x5F3759DF - 0x400000  # rsqrt seed magic for hv = 0.5*(v+eps)

    nc = bacc.Bacc(None, target_bir_lowering=False, debug=False)

    # All weight/token dram tensors are partition-major AND chunk-major:
    # each DMA'd unit (a weight column-chunk, a token tile) is CONTIGUOUS
    # per partition. This is what sets descriptor counts: the DGE emits one
    # descriptor per contiguous run, and with the old (P, K, H) layout a
    # 2MB W2 chunk was 2048 x 1KB runs (~15us of descriptor generation —
    # the real startup bottleneck). Chunk-major makes it 128 x 16KB runs.
    #   W1: (P, NC1, KD, CS)   [c, k] -> W1[k*128+p, c*CS+col]
    #   W2: (P, NC1, KH, CS)
    #   W3: (P, 2, KH, HD)
    #   tT: (P, n_tiles, KD, P)
    #   tT is declared flat 2D: 4D slices like [:, 1:6] were NOT merged into
    #   big runs by the AP optimizer (640 x 1KB descriptors, ~85GB/s); a 2D
    #   contiguous slice is guaranteed one run per partition.
    TTW = KD * P  # columns per token tile
    tT_d = nc.dram_tensor("tT", (P, n_tiles * TTW), bf16, kind="ExternalInput")
    w1a_d = nc.dram_tensor("W1a", (P, NC1, KD, CS), bf16, kind="ExternalInput")
    w2a_d = nc.dram_tensor("W2a", (P, NC1, KH, CS), bf16, kind="ExternalInput")
    w3a_d = nc.dram_tensor("W3a", (P, 2, KH, HD), bf16, kind="ExternalInput")
    cw_d = nc.dram_tensor("cw", (P, n_tiles), f32, kind="ExternalInput")
    out_d = nc.dram_tensor("out", (C, D), f32, kind="ExternalOutput")
    if two_seg:
        w1b_d = nc.dram_tensor("W1b", (P, NC1, KD, CS), bf16, kind="ExternalInput")
        w2b_d = nc.dram_tensor("W2b", (P, NC1, KH, CS), bf16, kind="ExternalInput")
        w3b_d = nc.dram_tensor("W3b", (P, 2, KH, HD), bf16, kind="ExternalInput")

    aff_d = {}
    for name, width in (
        ("b1", H), ("g1", H), ("be1", H),
        ("b2", H), ("g2", H), ("be2", H),
        ("b3", D),
    ):
        if affine[name]:
            aff_d[name] = nc.dram_tensor(name, (P, width), f32, kind="ExternalInput")

    with tile.TileContext(nc) as tc:
        with (
            tc.tile_pool(name="const", bufs=1) as const_pool,
            tc.tile_pool(name="hraw", bufs=2) as hraw_pool,
            tc.tile_pool(name="xg", bufs=2) as xg_pool,
            tc.tile_pool(name="hT", bufs=2) as hT_pool,
            tc.tile_pool(name="outp", bufs=2) as out_pool,
            tc.tile_pool(name="st", bufs=4) as st_pool,
            tc.tile_pool(name="acc", bufs=6, space="PSUM") as acc_pool,
            tc.tile_pool(name="tp", bufs=2, space="PSUM") as tp_pool,
        ):
            # ---- resident loads (segment A + shared) ----
            w1a_s = const_pool.tile((P, NC1, KD, CS), bf16)
            w2a_s = const_pool.tile((P, NC1, KH, CS), bf16)
            w3a_s = const_pool.tile((P, 2, KH, HD), bf16)
            tT_s = const_pool.tile((P, n_tiles * TTW), bf16)
            cw_s = const_pool.tile((P, n_tiles), f32)
            # Startup stream over the two independent DGE paths: sync (the
            # hardware DGE — NOTE scalar shares this same HW DGE, so a
            # scalar-queue DMA adds no bandwidth, it just steals from sync)
            # and gpsimd (software DGE). The W2-complete time gates tile 0's
            # L2, so everything not needed before ~45us (W3a, cw, trailing
            # tokens) goes BEHIND W2 on the queues. Segment-B weights are
            # NOT loaded here (deferred to j==1).
            # NOTE a DMA instruction only pushes descriptors (it completes in
            # <1us); per-RING service order == emission order, but the two
            # rings share the fabric. So: per-ring FIFO is the scheduler, and
            # the byte-prefix ahead of each item on its ring (plus the other
            # ring's concurrent load) sets its arrival time. Both rings are
            # balanced so all four W2 chunks land by ~41us; W3a (split in
            # halves across rings), cw and the segment-B W1/W3 go behind W2.
            # W2 c0 and c3 are split in k-halves across both rings: arrivals
            # then pace out as c0@~22, c1@~33, c2@~36, c3@~42, each ahead of
            # L2(0)'s 3.4us/chunk consumption which starts ~38 after the
            # 7-tile L1 prologue.
            PDp = PD + 1
            nc.sync.dma_start(tT_s[:, :TTW], tT_d[:, :TTW])
            nc.gpsimd.dma_start(w1a_s[:, 2], w1a_d[:, 2])
            nc.sync.dma_start(w1a_s[:, 0], w1a_d[:, 0])
            nc.gpsimd.dma_start(w1a_s[:, 3], w1a_d[:, 3])
            nc.sync.dma_start(w1a_s[:, 1], w1a_d[:, 1])
            nc.gpsimd.dma_start(tT_s[:, PDp * TTW:], tT_d[:, PDp * TTW:])
            nc.sync.dma_start(tT_s[:, TTW:PDp * TTW], tT_d[:, TTW:PDp * TTW])
            for c, q in ((0, nc.sync), (1, nc.gpsimd), (2, nc.sync), (3, nc.gpsimd)):
                q.dma_start(w2a_s[:, c], w2a_d[:, c])
            nc.sync.dma_start(w3a_s[:, 0], w3a_d[:, 0])
            nc.gpsimd.dma_start(w3a_s[:, 1], w3a_d[:, 1])
            nc.gpsimd.dma_start(cw_s[:], cw_d[:])
            if two_seg:
                w1b_s = const_pool.tile((P, NC1, KD, CS), bf16)
                w3b_s = const_pool.tile((P, 2, KH, HD), bf16)
                nc.sync.dma_start(w1b_s[:], w1b_d[:])
                nc.gpsimd.dma_start(w3b_s[:], w3b_d[:])

            identity = const_pool.tile((P, P), bf16)
            masks.make_identity(nc, identity[:])

            magic_t = const_pool.tile((P, 1), u32, name="magic_t")
            nc.vector.memset(magic_t[:], MAGIC_HALF)

            aff_s = {}
            for name in aff_d:
                width = aff_d[name].shape[1]
                row = const_pool.tile((P, width), f32, name=f"{name}_bcast")
                nc.sync.dma_start(row[:], aff_d[name][:])
                aff_s[name] = row

            def weights_for(i):
                if (not two_seg) or i < n_a:
                    return w1a_s, w2a_s, w3a_s
                return w1b_s, w2a_s, w3b_s

            # Can W2b's c1 chunk live in w1b_s? Only if every B tile's L1
            # (the w1b readers) is emitted before the c1 stream must go out
            # (i.e. before the first B tile's L2 at j == n_a).
            c1_in_w1b = two_seg and n_b <= NPRO - 1

            def w2_rhs_for(i, k, c):
                """Streaming rhs for L2 chunk c, k-block k of tile i.

                B tiles read chunk 0 from w1a_s's space and (usually) chunk 1
                from w1b_s's space: a (P,4,4,512) W1 tile and a (P,16,512) W2
                column-chunk are both 16KB/partition flat and line up as
                k*512+col == (k//4)*2048 + (k%4)*512 + col, and the W1
                spaces are dead once all L1s have run — so W2b c0/c1 stream
                there mid-kernel instead of in the late j==n_a-1 swap."""
                if (not two_seg) or i < n_a:
                    return w2a_s[:, c, k, :]
                if c == 0:
                    return w1a_s[:, k // KD, k % KD, :]
                if c == 1 and c1_in_w1b:
                    return w1b_s[:, k // KD, k % KD, :]
                return w2a_s[:, c, k, :]

            def dve_rsqrt(var_ap, eng=None):
                """rstd = 1/sqrt(var+eps) without an ACT-table switch.
                Seed y = bitcast(MAGIC_HALF - (bits(hv)>>1)) with hv =
                0.5*(var+eps), then two Newton steps y*(1.5 - hv*y^2).
                eng=nc.vector uses the fused custom DVE op; eng=nc.gpsimd
                runs the same math with plain ALU ops on the (prologue-idle)
                Pool engine so DVE keeps up with the 7-tile L1 prologue."""
                eng = eng or nc.vector
                on_dve = eng is nc.vector
                hv = st_pool.tile((P, 1), f32, tag="hv")
                eng.tensor_scalar(
                    out=hv[:], in0=var_ap, scalar1=0.5, scalar2=EPS * 0.5,
                    op0=ALU.mult, op1=ALU.add,
                )
                rstd = st_pool.tile((P, 1), f32, tag="rstd")
                eng.tensor_scalar(
                    out=rstd[:].bitcast(u32), in0=hv[:].bitcast(u32),
                    scalar1=1, scalar2=None, op0=ALU.logical_shift_right,
                )
                eng.tensor_tensor(
                    out=rstd[:].bitcast(u32), in0=magic_t[:],
                    in1=rstd[:].bitcast(u32), op=ALU.subtract,
                )
                nt = st_pool.tile((P, 1), f32, tag="nt")
                for _ in range(2):
                    eng.tensor_tensor(
                        out=nt[:], in0=hv[:], in1=rstd[:], op=ALU.mult
                    )
                    if on_dve:
                        nc.vector._custom_dve(
                            RECIPROCAL_APPROX_NR, out=rstd[:], in0=nt[:],
                            in1=rstd[:], s0=1.5,
                        )
                    else:
                        # y*(1.5 - nt*y) via plain ops: t = nt*y; t = 1.5-t;
                        # y = y*t
                        eng.tensor_tensor(
                            out=nt[:], in0=nt[:], in1=rstd[:], op=ALU.mult
                        )
                        eng.tensor_scalar(
                            out=nt[:], in0=nt[:], scalar1=-1.0, scalar2=1.5,
                            op0=ALU.mult, op1=ALU.add,
                        )
                        eng.tensor_tensor(
                            out=rstd[:], in0=rstd[:], in1=nt[:], op=ALU.mult
                        )
                return rstd

            def mm_ln_gelu(tile_i, lhsT_getter, n_k, rhs_get, nh, bname, gname, bename, xg_tag,
                           filler_after=None, drain_l1=False):
                """matmul (-> +b) -> LN -> (*g +be) -> gelu; returns xg tile.

                rhs_get(k, c) -> streaming-operand AP for k-block k, chunk c.
                filler_after: {chunk_idx: fn} — emit fn() after that chunk's
                matmuls (PE filler while a DMA-paced weight column streams)."""
                nch = nh // CS
                fast = not (affine[bname] or affine[gname] or affine[bename])
                # bf16 staging: only ever a gelu input (stats read f32 PSUM
                # directly); halves the pool so a deeper prologue fits SBUF
                hraw = hraw_pool.tile((P, nh), bf16 if fast else f32, tag="hraw")
                stats = st_pool.tile((P, nch, 6), f32, tag="stats")
                ps_list = []
                for c in range(nch):
                    ps = acc_pool.tile((P, CS), f32, name="ps_acc", tag="ps_acc")
                    for k in range(n_k):
                        nc.tensor.matmul(
                            ps[:],
                            lhsT_getter(k),
                            rhs_get(k, c),
                            start=(k == 0),
                            stop=(k == n_k - 1),
                        )
                    cs_sl = slice(c * CS, (c + 1) * CS)
                    if fast:
                        # stats read PSUM. Steady-state L1 keeps its PSUM
                        # until the gelu (no drain — keeps DVE free for the
                        # hT copies that gate the transpose ring); L2 drains
                        # to SBUF on DVE so banks free early. Prologue L1s
                        # drain FIRST (alternating ACT/DVE so neither engine
                        # backpressures PSUM recycling) and compute stats
                        # LATER in one wide 2048-col bn_stats over hraw
                        # (~1.3us vs 2.8us of per-chunk PSUM stats on DVE).
                        # GPSIMD cannot read PSUM on real HW.
                        nc.vector.bn_stats(stats[:, c, :], ps[:])
                        if drain_l1 and xg_tag == "xg1":
                            if c % 2 == 0:
                                nc.scalar.copy(hraw[:, cs_sl], ps[:])
                            else:
                                nc.vector.tensor_copy(hraw[:, cs_sl], ps[:])
                        elif xg_tag == "xg1":
                            ps_list.append(ps)
                        else:
                            nc.vector.tensor_copy(hraw[:, cs_sl], ps[:])
                    else:
                        nc.scalar.copy(hraw[:, cs_sl], ps[:])
                        if affine[bname]:
                            nc.vector.tensor_tensor(
                                out=hraw[:, cs_sl], in0=hraw[:, cs_sl],
                                in1=aff_s[bname][:, cs_sl], op=ALU.add,
                            )
                        nc.vector.bn_stats(stats[:, c, :], hraw[:, cs_sl])
                    if filler_after and c in filler_after:
                        filler_after[c]()
                mv = st_pool.tile((P, 2), f32, tag="mv")
                nc.vector.bn_aggr(mv[:], stats[:])
                # (Pool engine rejects TensorScalar at the ISA level, so the
                # whole LN chain stays on DVE)
                ln_eng = nc.vector
                rstd = dve_rsqrt(mv[:, 1:2], eng=ln_eng)
                negmr = st_pool.tile((P, 1), f32, tag="negmr")
                ln_eng.tensor_scalar(
                    out=negmr[:], in0=mv[:, 0:1], scalar1=rstd[:], scalar2=-1.0,
                    op0=ALU.mult, op1=ALU.mult,
                )
                xg = xg_pool.tile(
                    (P, nh), bf16, tag=xg_tag,
                    bufs=(NPRO if xg_tag == "xg1" else 2),
                )
                for c in range(nch):
                    cs_sl = slice(c * CS, (c + 1) * CS)
                    if fast:
                        # first piece of chunk 0 is narrow so the first PE
                        # transpose of this xg unblocks as early as possible
                        pieces = [(0, P), (P, CS)] if c == 0 else [(0, CS)]
                        for lo, hi in pieces:
                            if xg_tag == "xg1" and not drain_l1:
                                in_ap = ps_list[c][:, lo:hi]
                            else:
                                in_ap = hraw[:, c * CS + lo:c * CS + hi]
                            nc.scalar.activation(
                                xg[:, c * CS + lo:c * CS + hi], in_ap, AF.Gelu,
                                bias=negmr[:], scale=rstd[:],
                            )
                    else:
                        xn = hraw_pool.tile((P, CS), f32, name="xn", tag="xn")
                        nc.vector.tensor_scalar(
                            out=xn[:], in0=hraw[:, cs_sl],
                            scalar1=mv[:, 0:1], scalar2=rstd[:],
                            op0=ALU.subtract, op1=ALU.mult,
                        )
                        if affine[gname]:
                            nc.vector.tensor_tensor(
                                out=xn[:], in0=xn[:], in1=aff_s[gname][:, cs_sl],
                                op=ALU.mult,
                            )
                        if affine[bename]:
                            nc.vector.tensor_tensor(
                                out=xn[:], in0=xn[:], in1=aff_s[bename][:, cs_sl],
                                op=ALU.add,
                            )
                        nc.scalar.activation(xg[:, cs_sl], xn[:], AF.Gelu)
                return xg

            def transpose_to_hT(xg, nh, hT_tag, use_xbar=False, xq=None):
                """PE-transpose (P, nh) bf16 -> (P, nh//P, P) feature-major.

                hT1 (b-stage) drains on DVE, hT2 (c-stage) on ACT so neither
                engine's queue delays the other stage's PSUM->SBUF handoff."""
                nch = nh // CS
                hT = hT_pool.tile((P, nh // P, P), bf16, tag=hT_tag)
                if use_xbar:
                    # XBAR DMA transpose: hT[p,k,q] = xg[q,128k+p], ~1.8us on
                    # the DMA engine. Issued on the SYNC queue, not scalar:
                    # a DMA instruction occupies its engine until the wait
                    # clears, and on scalar that blocks the gelu stream.
                    # Sync mid-kernel only carries slack-tolerant stores and
                    # the W2b streams. Not used for early tiles whose XBAR
                    # would queue behind the 13MB startup stream.
                    (xq or nc.sync).dma_start_transpose(hT[:], xg[:])
                    return hT
                for c in range(nch):
                    pt = tp_pool.tile((P, CS), bf16, name="pt", tag="pt")
                    for j in range(CS // P):
                        b = c * (CS // P) + j
                        nc.tensor.transpose(
                            pt[:, j * P:(j + 1) * P],
                            xg[:, b * P:(b + 1) * P],
                            identity[:],
                        )
                    nc.vector.tensor_copy(
                        hT[:, c * (CS // P):(c + 1) * (CS // P), :], pt[:]
                    )
                return hT

            xg1 = {}
            xg2 = {}
            hT1 = {}
            hT2 = {}

            def stage_a(i):
                w1_s = weights_for(i)[0]
                xg1[i] = mm_ln_gelu(
                    i, lambda k: tT_s[:, i * TTW + k * P:i * TTW + (k + 1) * P],
                    KD, lambda k, c: w1_s[:, c, k, :], H,
                    "b1", "g1", "be1", "xg1", drain_l1=(i < NPRO - 1),
                )

            def stage_b(i, filler=None):
                h1T = hT1.pop(i)
                xg2[i] = mm_ln_gelu(
                    i, lambda k: h1T[:, k, :], KH,
                    lambda k, c: w2_rhs_for(i, k, c), H,
                    "b2", "g2", "be2", "xg2", filler_after=filler,
                )
                # last tile's chain gelu->XBAR->L3 is serial (nothing left
                # to overlap) while PE transposes pipeline per-chunk: keep
                # the tail (and startup-stream-blocked tile 0) on the PE
                # tail tiles use PE transposes: the XBAR path's chain
                # (all gelus -> whole-tile XBAR -> L3) is serial when no
                # other PE work remains, while PE transposes pipeline
                # per-chunk with the gelus
                hT2[i] = transpose_to_hT(xg2.pop(i), H, "hT2",
                                         use_xbar=(1 <= i <= n_tiles - 2))

            def stage_c(i):
                w3_s = weights_for(i)[2]
                h2T = hT2.pop(i)
                tok = slice(i * P, (i + 1) * P)
                outt = out_pool.tile((P, D), f32, tag="outt")
                # two D/2 halves: the first half's evict+store overlaps the
                # second half's matmuls (shrinks the end-of-kernel drain)
                for h in range(2):
                    dsl = slice(h * HD, (h + 1) * HD)
                    ps3 = acc_pool.tile((P, HD), f32, name="ps3", tag="ps_acc")
                    for k in range(KH):
                        nc.tensor.matmul(
                            ps3[:], h2T[:, k, :], w3_s[:, h, k, :],
                            start=(k == 0), stop=(k == KH - 1),
                        )
                    if affine["b3"]:
                        nc.vector.tensor_tensor(
                            out=outt[:, dsl], in0=ps3[:], in1=aff_s["b3"][:, dsl],
                            op=ALU.add,
                        )
                        nc.scalar.mul(outt[:, dsl], outt[:, dsl], cw_s[:, i:i + 1])
                    else:
                        nc.vector.tensor_scalar(
                            out=outt[:, dsl], in0=ps3[:],
                            scalar1=cw_s[:, i:i + 1], scalar2=None,
                            op0=ALU.mult, op1=ALU.bypass,
                        )
                    # alternate queues so the two stores overlap (matters for
                    # the end-of-kernel drain of the final tile)
                    (nc.gpsimd, nc.sync)[h].dma_start(out_d[tok, dsl], outt[:, dsl])

            # prologue: queue NPRO stage-A tiles so the PE has L1 work while
            # the 8MB W2 load is still streaming in (all but the last drain
            # their PSUM to SBUF; the last prologue tile keeps PSUM)
            next_a = NPRO
            for i in range(next_a):
                stage_a(i)
                # interleave the first two hT1 transposes into the prologue:
                # their DVE drains then run BEFORE the later tiles' LN
                # chains in the in-order DVE stream, so hT1(0) (which gates
                # L2(0)) is ready ~5us earlier
                if i == 2:
                    hT1[0] = transpose_to_hT(xg1.pop(0), H, "hT1")
                if i == 3 and n_tiles > 1:
                    hT1[1] = transpose_to_hT(xg1.pop(1), H, "hT1")
            if 0 not in hT1:
                hT1[0] = transpose_to_hT(xg1.pop(0), H, "hT1")

            w2b_c0_done = not two_seg
            w2b_c1_done = not (two_seg and c1_in_w1b)
            for j in range(n_tiles):
                if j + 1 < n_tiles and j + 1 not in hT1:
                    hT1[j + 1] = transpose_to_hT(
                        xg1.pop(j + 1), H, "hT1",
                        use_xbar=(2 <= j + 1 <= n_tiles - 2),
                    )
                stage_b(j)
                # B-tile L1s are additionally held until j >= 2 so their
                # w1b dependency (ring tail, ~55us) can't stall the queue
                if (next_a < n_tiles and next_a <= j + NPRO
                        and (next_a < n_a or j >= 2)):
                    stage_a(next_a)
                    next_a += 1
                # W2b c0 (and usually c1) stream into the dead W1 spaces.
                # CAREFUL: a DMA instruction's semaphore wait BLOCKS its
                # whole in-order queue (out-stores behind it → outt-buffer
                # recycling → PE stall), so emit these only at a j where the
                # wait (all L1 readers of that W1 space done) will already
                # have cleared when the instruction reaches the queue head
                # (the queue head trails by ~2 tiles of out-stores).
                if (two_seg and not w2b_c0_done and next_a > n_a - 1
                        and j >= n_a - 3):
                    nc.gpsimd.dma_start(w1a_s[:], w2b_d[:, 0])
                    w2b_c0_done = True
                if (not w2b_c1_done and next_a > n_tiles - 1
                        and j >= n_a - 2):
                    nc.sync.dma_start(w1b_s[:], w2b_d[:, 1])
                    w2b_c1_done = True
                # remaining W2b chunks overwrite w2a_s once their last
                # segment-A L2 readers are issued (the 4MB late swap)
                if two_seg and j == n_a - 1:
                    late = [2, 3] if c1_in_w1b else [1, 2, 3]
                    for c in late:
                        q = (nc.sync, nc.gpsimd)[c % 2]
                        q.dma_start(w2a_s[:, c], w2b_d[:, c])
                if j >= 1:
                    stage_c(j - 1)
            stage_c(n_tiles - 1)

    nc.compile()
    return nc


def kernel(**inputs):
    global last_exec_time_ns
    import ml_dtypes

    from concourse import bass_utils

    inp = {k: np.asarray(v) for k, v in inputs.items()}
    x = inp["x"].astype(np.float32, copy=False)
    B, S, d = x.shape
    T = B * S
    t = x.reshape(T, d)

    top2, topv = _route(t, inp["Wg1"], inp["bg1"], inp["Wg2"], inp["bg2"])

    idx_per_e = []
    w_per_e = []
    for e in range(E):
        sel = np.nonzero(top2 == e)
        idx_per_e.append(sel[0])
        w_per_e.append(topv[sel].astype(np.float32))

    affine = {
        "b1": not np.all(inp["b1"] == 0.0),
        "g1": not np.all(inp["g1"] == 1.0),
        "be1": not np.all(inp["be1"] == 0.0),
        "b2": not np.all(inp["b2"] == 0.0),
        "g2": not np.all(inp["g2"] == 1.0),
        "be2": not np.all(inp["be2"] == 0.0),
        "b3": not np.all(inp["b3"] == 0.0),
    }
    any_affine = any(affine.values())

    tiles_e = [int(math.ceil(len(ix) / P)) for ix in idx_per_e]
    if any_affine:
        # affine params are per-expert; keep one expert per core
        sA, sB = max(max(tiles_e), 1), 0
        assign = [(1, 0)] * E
    else:
        sA, sB, assign = _pack_segments(tiles_e)

    # build slot lists: each slot = (expert, first_piece, n_pieces)
    slotsA, slotsB = [], []
    for e in range(E):
        a_e, b_e = assign[e]
        pos = 0
        nt = tiles_e[e]
        for _ in range(a_e):
            take = max(0, min(sA, nt - pos))
            slotsA.append((e, pos, take))
            pos += take
        for _ in range(b_e):
            take = max(0, min(sB, nt - pos))
            slotsB.append((e, pos, take))
            pos += take
    while len(slotsA) < E:
        slotsA.append((None, 0, 0))
    while len(slotsB) < E:
        slotsB.append((None, 0, 0))

    n_tiles = sA + sB
    C = n_tiles * P
    CS = 512
    bf = ml_dtypes.bfloat16
    # chunk-major layouts (see _build_program): each DMA'd unit contiguous
    zW1 = np.zeros((P, H // CS, D // P, CS), bf)
    zW2 = np.zeros((P, H // CS, H // P, CS), bf)
    zW3 = np.zeros((P, 2, H // P, D // 2), bf)

    def slot_tokens(slot, s_cap):
        """token columns (D, s_cap*P) f32 + cw (s_cap*P,) for one slot."""
        e, pos, take = slot
        tt = np.zeros((D, s_cap * P), np.float32)
        cw = np.zeros((s_cap * P,), np.float32)
        if e is not None and take > 0:
            lo = pos * P
            hi = min(len(idx_per_e[e]), (pos + take) * P)
            n = hi - lo
            tt[:, :n] = t[idx_per_e[e][lo:hi]].T
            cw[:n] = w_per_e[e][lo:hi]
        return tt, cw

    def expert_w(e, which):
        if e is None:
            return (zW1, zW2, zW3)[which]
        w = (inp["W1"], inp["W2"], inp["W3"])[which][e]
        kk = (D // P, H // P, H // P)[which]
        cw_ = (CS, CS, D // 2)[which]
        nch = w.shape[1] // cw_
        # partition-major AND chunk-major: (P, nch, kk, cw) — the DMA'd
        # unit (one column-chunk) is contiguous per partition
        return np.ascontiguousarray(
            np.asarray(w).reshape(kk, P, nch, cw_).transpose(1, 2, 0, 3).astype(bf)
        )

    in_maps = []
    for c in range(E):
        ttA, cwA = slot_tokens(slotsA[c], sA)
        eA = slotsA[c][0]
        if sB > 0:
            ttB, cwB = slot_tokens(slotsB[c], sB)
            tt = np.concatenate([ttA, ttB], axis=1)
            cw = np.concatenate([cwA, cwB])
        else:
            tt, cw = ttA, cwA
        m = {
            # flat (P, n_tiles*KD*P): one token tile contiguous per partition
            "tT": np.ascontiguousarray(
                tt.reshape(D // P, P, n_tiles, P).transpose(1, 2, 0, 3)
            ).astype(bf).reshape(P, -1),
            "W1a": expert_w(eA, 0),
            "W2a": expert_w(eA, 1),
            "W3a": expert_w(eA, 2),
            "cw": np.ascontiguousarray(cw.reshape(n_tiles, P).T).astype(np.float32),
        }
        if sB > 0:
            eB = slotsB[c][0]
            m["W1b"] = expert_w(eB, 0)
            m["W2b"] = expert_w(eB, 1)
            m["W3b"] = expert_w(eB, 2)
        for name in ("b1", "g1", "be1", "b2", "g2", "be2", "b3"):
            if affine[name]:
                row = np.asarray(inp[name][eA if eA is not None else 0], np.float32)
                m[name] = np.ascontiguousarray(np.broadcast_to(row, (P, row.shape[0])))
        in_maps.append(m)

    nc = _build_program(sA, sB, affine)

    trace = bool(os.environ.get("KERNEL_TRACE"))
    if trace:
        try:
            from antenv import axon_hooks as _ah  # noqa: F401
        except ImportError:
            trace = False
    try:
        res = bass_utils.run_bass_kernel_spmd(
            nc, in_maps, core_ids=list(range(E)), trace=trace
        )
    except Exception:
        if not trace:
            raise
        res = bass_utils.run_bass_kernel_spmd(
            nc, in_maps, core_ids=list(range(E)), trace=False
        )
    last_exec_time_ns = getattr(res, "exec_time_ns", None)

    out_full = np.zeros((T, D), np.float32)
    for c in range(E):
        o = np.asarray(res.results[c]["out"], np.float32)
        for si, (slot, s_cap, base) in enumerate(
            ((slotsA[c], sA, 0), (slotsB[c], sB, sA * P))
        ):
            e, pos, take = slot
            if e is None or take == 0:
                continue
            lo = pos * P
            hi = min(len(idx_per_e[e]), (pos + take) * P)
            n = hi - lo
            out_full[idx_per_e[e][lo:hi]] += o[base:base + n]
    return out_full.reshape(B, S, D).astype(np.float32)

